# revision 7
# baseline (speedup 1.0000x reference)
"""AllPoleDigitalFilter Trainium2 kernel.

y[t] = K_int[t]*x[t] - sum_{i=1..30} a_int[t,i] * y[t-i]
with a_int/K_int linearly interpolated from frame coefficients (frame period 80).

Strategy (per core, 8 of 64 batch sequences):
 - Overlap-save chunking: each sequence split into 16 chunks of L=1000 samples;
   each chunk instance recomputes a W=240-sample warmup from zero state (the
   filter's homogeneous response decays below 1e-10 within 240 samples for
   these coefficients: sum_i |a_i| <= 0.63).
 - 128 partitions = 128 chunk instances (8 seqs x 16 chunks). The order-30
   recurrence runs as one scalar_tensor_tensor (+accumulator read) per sample
   on the Vector engine:
     ybuf[p, 30+j] = sum_d A[p, j, d] * ybuf[p, j+d],  d in [0, 31)
   where A[p,j,d] = -a_int[t, 30-d] for d<30 and A[p,j,30] = K_int*x; ybuf
   slots not yet computed are prefilled with 1.0 so the last window element
   contributes the input term, and the accumulator result overwrites it.
 - The A coefficient stream (31 floats per sample) is interpolated tile by
   tile on the GpSimd engine from per-frame coefficients via broadcast /
   reversed access patterns, running ahead of the Vector chain.
"""
import numpy as np

B, T = 64, 16000
NSEQ = 8           # sequences per core
NCORE = 8
W = 240            # warmup samples per chunk
L = 1000           # chunk payload
WP = W + L         # window samples per instance (1240)
NFR = 17           # frames stored per partition
NU = 32            # half-frame slots stored per partition
NFP = 202          # padded frame count in dram
XP_LEN = W + T     # 16240
TILES = [240, 240, 240, 240, 280]

_prog = None


def _build_program():
    import concourse.bacc as bacc
    import concourse.mybir as mybir
    import concourse.bass as bass
    from concourse.tile import TileContext

    f32 = mybir.dt.float32
    AP = bass.AP
    mult = mybir.AluOpType.mult
    add = mybir.AluOpType.add
    sub = mybir.AluOpType.subtract

    nc = bacc.Bacc("TRN2", target_bir_lowering=False, name="apdf")
    xp_d = nc.dram_tensor("xp", (NSEQ, XP_LEN), f32, kind="ExternalInput")
    af_d = nc.dram_tensor("af", (NSEQ, NFP, 31), f32, kind="ExternalInput")
    ftab_d = nc.dram_tensor("ftabN", (128, 280), f32, kind="ExternalInput")
    y_d = nc.dram_tensor("y", (NSEQ, T), f32, kind="ExternalOutput")

    # partition p = parity*64 + s*8 + k ; chunk m = 2*k + parity
    # window start w0 = 1000*m - W ; phase phi = 40*parity
    # base frame n0: parity 0: 25k - 3 (k=0 clamped to 0), parity 1: 25k + 9

    with TileContext(nc) as tc:
        with tc.tile_pool(name="sbuf", bufs=1) as pool, \
             tc.tile_pool(name="atiles", bufs=2) as apool:
            fr = pool.tile([128, NFR, 31], f32)
            frh = pool.tile([128, NU, 31], f32)
            frh1 = pool.tile([128, NU, 31], f32)
            dfh = pool.tile([128, NU, 31], f32)
            frhN = pool.tile([128, NU, 31], f32)
            xwin = pool.tile([128, WP], f32)
            ybuf = pool.tile([128, 30 + WP], f32)
            ftab = pool.tile([128, 280], f32)
            t2 = pool.tile([128, 280], f32)
            t3 = pool.tile([128, 280], f32)
            scr = pool.tile([128, 31], f32)

            # ---------------- input DMAs ----------------
            nc.sync.dma_start(out=ftab[:], in_=ftab_d[:])

            # frame coefficients (issued first: they gate the A generation)
            # zero the whole tile first: parity-0 k=0 keeps zeros in its first
            # 3 (clamped, pre-t=0) local frames
            nc.gpsimd.memset(fr[:].rearrange("p n d -> p (n d)"), 0.0)
            fr4 = fr[:].rearrange("(c s k) n d -> c s k (n d)", c=2, s=8, k=8)
            for s in range(NSEQ):
                # parity 0, k >= 1: n0 = 25k - 3
                nc.sync.dma_start(
                    out=fr4[0, s, 1:8],
                    in_=AP(tensor=af_d, offset=s * NFP * 31 + 22 * 31,
                           ap=[[25 * 31, 7], [1, NFR * 31]]),
                )
                # parity 0, k = 0 (clamped): local frames [3:17) <- frames [0:14)
                nc.sync.dma_start(
                    out=fr4[0, s, 0:1, 3 * 31:],
                    in_=AP(tensor=af_d, offset=s * NFP * 31,
                           ap=[[14 * 31, 1], [1, 14 * 31]]),
                )
                # parity 1: n0 = 25k + 9
                nc.sync.dma_start(
                    out=fr4[1, s],
                    in_=AP(tensor=af_d, offset=s * NFP * 31 + 9 * 31,
                           ap=[[25 * 31, 8], [1, NFR * 31]]),
                )


            # x windows: partition (parity, s, k) <- xp[s, 1000*(2k+parity) : +WP]
            xw4 = xwin[:].rearrange("(c s k) j -> c s k j", c=2, s=8, k=8)
            for par in (0, 1):
                for s in range(NSEQ):
                    src = AP(tensor=xp_d, offset=s * XP_LEN + 1000 * par,
                             ap=[[2000, 8], [1, WP]])
                    nc.sync.dma_start(out=xw4[par, s], in_=src)

            # ------------- half-frame expansion (gpsimd) -------------
            # frh[p, u]  = fr[p, floor((40u+phi)/80)]
            # frh1[p, u] = fr[p, floor((40u+phi)/80) + 1]
            for buf in (frh, frh1, dfh, frhN):
                nc.gpsimd.memset(buf[:].rearrange("p u d -> p (u d)"), 0.0)
            # parity 0 (phi=0): even u <- fr[v], odd u <- fr[v]
            nc.gpsimd.tensor_copy(out=frh[0:64, 0:32:2, :], in_=fr[0:64, 0:16, :])
            nc.gpsimd.tensor_copy(out=frh[0:64, 1:32:2, :], in_=fr[0:64, 0:16, :])
            nc.gpsimd.tensor_copy(out=frh1[0:64, 0:32:2, :], in_=fr[0:64, 1:17, :])
            nc.gpsimd.tensor_copy(out=frh1[0:64, 1:32:2, :], in_=fr[0:64, 1:17, :])
            # parity 1 (phi=40): even u <- fr[v], odd u <- fr[v+1]
            nc.gpsimd.tensor_copy(out=frh[64:128, 0:32:2, :], in_=fr[64:128, 0:16, :])
            nc.gpsimd.tensor_copy(out=frh[64:128, 1:32:2, :], in_=fr[64:128, 1:17, :])
            nc.gpsimd.tensor_copy(out=frh1[64:128, 0:32:2, :], in_=fr[64:128, 1:17, :])
            nc.gpsimd.tensor_copy(out=frh1[64:128, 1:31:2, :], in_=fr[64:128, 2:17, :])
            nc.gpsimd.tensor_tensor(
                out=dfh[:].rearrange("p u d -> p (u d)"),
                in0=frh1[:].rearrange("p u d -> p (u d)"),
                in1=frh[:].rearrange("p u d -> p (u d)"),
                op=sub,
            )
            nc.gpsimd.tensor_scalar_mul(
                frhN[:].rearrange("p u d -> p (u d)"),
                frh[:].rearrange("p u d -> p (u d)"),
                -1.0,
            )

            # ---------------- y buffer init ----------------
            nc.gpsimd.memset(ybuf[:, 0:30], 0.0)
            nc.gpsimd.memset(ybuf[:, 30:], 1.0)

            # ------------- tiled A generation (gpsimd) + stepping (vector) ----
            j0 = 0
            u0 = 0
            for ts in TILES:
                nu_t = ts // 40
                atile = apool.tile([128, 280, 31], f32, tag="A")
                av = atile[:, 0:ts, 0:30].rearrange("p (u r) d -> p u r d", r=40)
                ftv = ftab[:, 0:ts].rearrange("p (u r) -> p u r", r=40)
                # pass 1: A[:, :, 0:30] = ftab (bcast d) * dfh (bcast r, rev d)
                nc.gpsimd.tensor_tensor(
                    out=av,
                    in0=ftv[:, :, :, None].broadcast_to([128, nu_t, 40, 30]),
                    in1=dfh[:, u0 : u0 + nu_t, None, 30:0:-1].broadcast_to(
                        [128, nu_t, 40, 30]),
                    op=mult,
                )
                # pass 2: A += frhN (bcast r, rev d)
                nc.gpsimd.tensor_tensor(
                    out=av,
                    in0=av,
                    in1=frhN[:, u0 : u0 + nu_t, None, 30:0:-1].broadcast_to(
                        [128, nu_t, 40, 30]),
                    op=add,
                )
                # xg column: Kint = K - ftab*dK ; A[:, :, 30] = Kint * xwin
                t2v = t2[:, 0:ts].rearrange("p (u r) -> p u r", r=40)
                t3v = t3[:, 0:ts].rearrange("p (u r) -> p u r", r=40)
                nc.gpsimd.tensor_tensor(
                    out=t2v,
                    in0=ftv,
                    in1=dfh[:, u0 : u0 + nu_t, 0][:, :, None].broadcast_to(
                        [128, nu_t, 40]),
                    op=mult,
                )
                nc.gpsimd.tensor_tensor(
                    out=t3v,
                    in0=frh[:, u0 : u0 + nu_t, 0][:, :, None].broadcast_to(
                        [128, nu_t, 40]),
                    in1=t2v,
                    op=sub,
                )
                nc.gpsimd.tensor_tensor(
                    out=atile[:, 0:ts, 30],
                    in0=t3[:, 0:ts],
                    in1=xwin[:, j0 : j0 + ts],
                    op=mult,
                )

                # stepping over this tile (vector engine serial chain)
                for jl in range(ts):
                    j = j0 + jl
                    nc.vector.scalar_tensor_tensor(
                        out=scr[:],
                        in0=atile[:, jl, :],
                        scalar=0.0,
                        in1=ybuf[:, j : j + 31],
                        op0=mybir.AluOpType.bypass,
                        op1=mult,
                        accum_out=ybuf[:, 30 + j : 31 + j],
                    )
                j0 += ts
                u0 += nu_t

            # ---------------- output DMAs ----------------
            yv = ybuf[:, 30 + W : 30 + W + L].rearrange(
                "(c s k) j -> c s k j", c=2, s=8, k=8)
            for par in (0, 1):
                for s in range(NSEQ):
                    dst = AP(tensor=y_d, offset=s * T + 1000 * par,
                             ap=[[2000, 8], [1, L]])
                    nc.sync.dma_start(out=dst, in_=yv[par, s])

    nc.compile()
    return nc


def _get_prog():
    global _prog
    if _prog is None:
        _prog = _build_program()
    return _prog


def _host_inputs(x, a):
    x = np.ascontiguousarray(x, dtype=np.float32)
    a = np.ascontiguousarray(a, dtype=np.float32)
    xp = np.zeros((B, XP_LEN), np.float32)
    xp[:, W:] = x
    af = np.zeros((B, NFP, 31), np.float32)
    af[:, :200] = a
    af[:, 200] = a[:, 199]
    jl = np.arange(280)
    ftabN = np.zeros((128, 280), np.float32)
    ftabN[0:64] = -((jl % 80) / 80.0)
    ftabN[64:128] = -(((jl + 40) % 80) / 80.0)
    in_maps = []
    for c in range(NCORE):
        sl = slice(c * NSEQ, (c + 1) * NSEQ)
        in_maps.append({"xp": xp[sl], "af": af[sl], "ftabN": ftabN})
    return in_maps


def kernel(x, a):
    from concourse import bass_utils

    nc = _get_prog()
    in_maps = _host_inputs(x, a)
    res = bass_utils.run_bass_kernel_spmd(nc, in_maps, core_ids=list(range(NCORE)))
    out = np.empty((B, T), np.float32)
    for c in range(NCORE):
        out[c * NSEQ : (c + 1) * NSEQ] = res.results[c]["y"]
    return out


# revision 9
# speedup vs baseline: 1.0137x; 1.0137x over previous
"""AllPoleDigitalFilter Trainium2 kernel.

y[t] = K_int[t]*x[t] - sum_{i=1..30} a_int[t,i] * y[t-i]
with a_int/K_int linearly interpolated from frame coefficients (frame period 80).

Strategy (per core, 8 of 64 batch sequences):
 - Overlap-save chunking: each sequence split into 16 chunks of L=1000 samples;
   each chunk instance recomputes a W=240-sample warmup from zero state (the
   filter's homogeneous response decays below 1e-10 within 240 samples for
   these coefficients: sum_i |a_i| <= 0.63).
 - 128 partitions = 128 chunk instances (8 seqs x 16 chunks). The order-30
   recurrence runs as one scalar_tensor_tensor (+accumulator read) per sample
   on the Vector engine:
     ybuf[p, 30+j] = sum_d A[p, j, d] * ybuf[p, j+d],  d in [0, 31)
   where A[p,j,d] = -a_int[t, 30-d] for d<30 and A[p,j,30] = K_int*x; ybuf
   slots not yet computed are prefilled with 1.0 so the last window element
   contributes the input term, and the accumulator result overwrites it.
 - The A coefficient stream (31 floats per sample) is interpolated tile by
   tile on the GpSimd engine from per-frame coefficients via broadcast /
   reversed access patterns, running ahead of the Vector chain.
"""
import numpy as np

B, T = 64, 16000
NSEQ = 8           # sequences per core
NCORE = 8
W = 240            # warmup samples per chunk
L = 1000           # chunk payload
WP = W + L         # window samples per instance (1240)
NFR = 17           # frames stored per partition
NU = 32            # half-frame slots stored per partition
NFP = 202          # padded frame count in dram
XP_LEN = W + T     # 16240
TILES = [240, 240, 240, 240, 280]

_prog = None


def _build_program():
    import concourse.bacc as bacc
    import concourse.mybir as mybir
    import concourse.bass as bass
    from concourse.tile import TileContext

    f32 = mybir.dt.float32
    AP = bass.AP
    mult = mybir.AluOpType.mult
    add = mybir.AluOpType.add
    sub = mybir.AluOpType.subtract

    nc = bacc.Bacc("TRN2", target_bir_lowering=False, name="apdf",
                   detect_race_conditions=False)
    xp_d = nc.dram_tensor("xp", (NSEQ, XP_LEN), f32, kind="ExternalInput")
    af_d = nc.dram_tensor("af", (NSEQ, NFP, 31), f32, kind="ExternalInput")
    ftab_d = nc.dram_tensor("ftabN", (128, 280), f32, kind="ExternalInput")
    y_d = nc.dram_tensor("y", (NSEQ, T), f32, kind="ExternalOutput")

    # partition p = parity*64 + s*8 + k ; chunk m = 2*k + parity
    # window start w0 = 1000*m - W ; phase phi = 40*parity
    # base frame n0: parity 0: 25k - 3 (k=0 clamped to 0), parity 1: 25k + 9

    with TileContext(nc) as tc:
        with tc.tile_pool(name="sbuf", bufs=1) as pool, \
             tc.tile_pool(name="atiles", bufs=3) as apool:
            fr = pool.tile([128, NFR, 31], f32)
            frh = pool.tile([128, NU, 31], f32)
            frh1 = pool.tile([128, NU, 31], f32)
            dfh = pool.tile([128, NU, 31], f32)
            frhN = pool.tile([128, NU, 31], f32)
            xwin = pool.tile([128, WP], f32)
            ybuf = pool.tile([128, 30 + WP], f32)
            ftab = pool.tile([128, 280], f32)
            t2 = pool.tile([128, 280], f32)
            t3 = pool.tile([128, 280], f32)
            scr = pool.tile([128, 31], f32)

            # ---------------- input DMAs ----------------
            nc.sync.dma_start(out=ftab[:], in_=ftab_d[:])

            # frame coefficients (issued first: they gate the A generation)
            # zero first 3 local frames of parity-0 partitions: k=0 (clamped)
            # keeps zeros there; k>=1 partitions get overwritten by their DMA
            nc.gpsimd.memset(fr[0:64, 0:3, :].rearrange("p n d -> p (n d)"), 0.0)
            fr4 = fr[:].rearrange("(c s k) n d -> c s k (n d)", c=2, s=8, k=8)
            for s in range(NSEQ):
                # parity 0, k >= 1: n0 = 25k - 3
                nc.sync.dma_start(
                    out=fr4[0, s, 1:8],
                    in_=AP(tensor=af_d, offset=s * NFP * 31 + 22 * 31,
                           ap=[[25 * 31, 7], [1, NFR * 31]]),
                )
                # parity 0, k = 0 (clamped): frames [3:17) <- dram [0:14)
                nc.sync.dma_start(
                    out=fr4[0, s, 0:1, 3 * 31:],
                    in_=AP(tensor=af_d, offset=s * NFP * 31,
                           ap=[[14 * 31, 1], [1, 14 * 31]]),
                )
                # parity 1: n0 = 25k + 9
                nc.sync.dma_start(
                    out=fr4[1, s],
                    in_=AP(tensor=af_d, offset=s * NFP * 31 + 9 * 31,
                           ap=[[25 * 31, 8], [1, NFR * 31]]),
                )

            # x windows: partition (parity, s, k) <- xp[s, 1000*(2k+parity) : +WP]
            xw4 = xwin[:].rearrange("(c s k) j -> c s k j", c=2, s=8, k=8)
            for par in (0, 1):
                for s in range(NSEQ):
                    xsrc = AP(tensor=xp_d, offset=s * XP_LEN + 1000 * par,
                              ap=[[2000, 8], [1, WP]])
                    nc.sync.dma_start(out=xw4[par, s], in_=xsrc)

            # ------------- half-frame expansion (gpsimd) -------------
            # frh[p, u]  = fr[p, floor((40u+phi)/80)]
            # frh1[p, u] = fr[p, floor((40u+phi)/80) + 1]
            for buf in (frh, frh1, dfh, frhN):
                nc.vector.memset(buf[:].rearrange("p u d -> p (u d)"), 0.0)
            # parity 0 (phi=0): even u <- fr[v], odd u <- fr[v]
            nc.vector.tensor_copy(out=frh[0:64, 0:32:2, :], in_=fr[0:64, 0:16, :])
            nc.vector.tensor_copy(out=frh[0:64, 1:32:2, :], in_=fr[0:64, 0:16, :])
            nc.vector.tensor_copy(out=frh1[0:64, 0:32:2, :], in_=fr[0:64, 1:17, :])
            nc.vector.tensor_copy(out=frh1[0:64, 1:32:2, :], in_=fr[0:64, 1:17, :])
            # parity 1 (phi=40): even u <- fr[v], odd u <- fr[v+1]
            nc.vector.tensor_copy(out=frh[64:128, 0:32:2, :], in_=fr[64:128, 0:16, :])
            nc.vector.tensor_copy(out=frh[64:128, 1:32:2, :], in_=fr[64:128, 1:17, :])
            nc.vector.tensor_copy(out=frh1[64:128, 0:32:2, :], in_=fr[64:128, 1:17, :])
            nc.vector.tensor_copy(out=frh1[64:128, 1:31:2, :], in_=fr[64:128, 2:17, :])
            nc.vector.tensor_tensor(
                out=dfh[:].rearrange("p u d -> p (u d)"),
                in0=frh1[:].rearrange("p u d -> p (u d)"),
                in1=frh[:].rearrange("p u d -> p (u d)"),
                op=sub,
            )
            nc.vector.tensor_scalar_mul(
                frhN[:].rearrange("p u d -> p (u d)"),
                frh[:].rearrange("p u d -> p (u d)"),
                -1.0,
            )

            # ---------------- y buffer init ----------------
            nc.gpsimd.memset(ybuf[:, 0:30], 0.0)
            nc.gpsimd.memset(ybuf[:, 30:], 1.0)

            # ------------- tiled A generation + stepping (all vector) ----
            j0 = 0
            u0 = 0
            for ts in TILES:
                nu_t = ts // 40
                atile = apool.tile([128, 280, 31], f32, tag="A")
                av = atile[:, 0:ts, 0:30].rearrange("p (u r) d -> p u r d", r=40)
                ftv = ftab[:, 0:ts].rearrange("p (u r) -> p u r", r=40)
                # pass 1: A[:, :, 0:30] = ftab (bcast d) * dfh (bcast r, rev d)
                nc.vector.tensor_tensor(
                    out=av,
                    in0=ftv[:, :, :, None].broadcast_to([128, nu_t, 40, 30]),
                    in1=dfh[:, u0 : u0 + nu_t, None, 30:0:-1].broadcast_to(
                        [128, nu_t, 40, 30]),
                    op=mult,
                )
                # pass 2: A += frhN (bcast r, rev d)
                nc.vector.tensor_tensor(
                    out=av,
                    in0=av,
                    in1=frhN[:, u0 : u0 + nu_t, None, 30:0:-1].broadcast_to(
                        [128, nu_t, 40, 30]),
                    op=add,
                )
                # xg column: Kint = K - ftab*dK ; A[:, :, 30] = Kint * xwin
                t2v = t2[:, 0:ts].rearrange("p (u r) -> p u r", r=40)
                t3v = t3[:, 0:ts].rearrange("p (u r) -> p u r", r=40)
                nc.vector.tensor_tensor(
                    out=t2v,
                    in0=ftv,
                    in1=dfh[:, u0 : u0 + nu_t, 0][:, :, None].broadcast_to(
                        [128, nu_t, 40]),
                    op=mult,
                )
                nc.vector.tensor_tensor(
                    out=t3v,
                    in0=frh[:, u0 : u0 + nu_t, 0][:, :, None].broadcast_to(
                        [128, nu_t, 40]),
                    in1=t2v,
                    op=sub,
                )
                nc.vector.tensor_tensor(
                    out=atile[:, 0:ts, 30],
                    in0=t3[:, 0:ts],
                    in1=xwin[:, j0 : j0 + ts],
                    op=mult,
                )

                # stepping over this tile (vector engine serial chain)
                for jl in range(ts):
                    j = j0 + jl
                    nc.vector.scalar_tensor_tensor(
                        out=scr[:],
                        in0=atile[:, jl, :],
                        scalar=0.0,
                        in1=ybuf[:, j : j + 31],
                        op0=mybir.AluOpType.bypass,
                        op1=mult,
                        accum_out=ybuf[:, 30 + j : 31 + j],
                    )
                j0 += ts
                u0 += nu_t

            # ---------------- output DMAs ----------------
            yv = ybuf[:, 30 + W : 30 + W + L].rearrange(
                "(c s k) j -> c s k j", c=2, s=8, k=8)
            for par in (0, 1):
                for s in range(NSEQ):
                    dst = AP(tensor=y_d, offset=s * T + 1000 * par,
                             ap=[[2000, 8], [1, L]])
                    nc.sync.dma_start(out=dst, in_=yv[par, s])

    nc.compile()
    return nc


def _get_prog():
    global _prog
    if _prog is None:
        _prog = _build_program()
    return _prog


def _host_inputs(x, a):
    x = np.ascontiguousarray(x, dtype=np.float32)
    a = np.ascontiguousarray(a, dtype=np.float32)
    xp = np.zeros((B, XP_LEN), np.float32)
    xp[:, W:] = x
    af = np.zeros((B, NFP, 31), np.float32)
    af[:, :200] = a
    af[:, 200] = a[:, 199]
    jl = np.arange(280)
    ftabN = np.zeros((128, 280), np.float32)
    ftabN[0:64] = -((jl % 80) / 80.0)
    ftabN[64:128] = -(((jl + 40) % 80) / 80.0)
    in_maps = []
    for c in range(NCORE):
        sl = slice(c * NSEQ, (c + 1) * NSEQ)
        in_maps.append({"xp": xp[sl], "af": af[sl], "ftabN": ftabN})
    return in_maps


def kernel(x, a):
    from concourse import bass_utils

    nc = _get_prog()
    in_maps = _host_inputs(x, a)
    res = bass_utils.run_bass_kernel_spmd(nc, in_maps, core_ids=list(range(NCORE)))
    out = np.empty((B, T), np.float32)
    for c in range(NCORE):
        out[c * NSEQ : (c + 1) * NSEQ] = res.results[c]["y"]
    return out
